# revision 1
# baseline (speedup 1.0000x reference)
"""Trainium2 Bass kernel for nn_NodeProcessor (GNN message passing).

Strategy (8 NeuronCores, SPMD, no collectives):
  - Host sorts edges by destination node and shards NODES (6250/core);
    each core receives exactly the edges destined to its node shard, so no
    cross-core reduction is needed.
  - On device, segment-sum is computed per 128-node tile as a sequence of
    128-edge-chunk matmuls accumulating in PSUM:
        agg_T[f, n] += sum_e edge_chunk[e, f] * S[e, n],
    where S[e, n] = (j_rel[e] == n) is a one-hot selection matrix built by
    an is_equal compare against a constant iota.  Edges are pre-sorted, so
    each chunk belongs to one node tile and spans few nodes: chunk 0 of a
    tile writes the full 128-wide region (start=True clears the
    accumulator), later chunks compare/accumulate only a W=32-wide window
    at a host-baked column offset.
  - Edge payload and the x MLP input are fp8 e3m4 (loose 2e-2 tolerance;
    measured end-to-end rel err ~1e-2): halves the dominant HBM traffic.
    S matrices stay bf16 and are built with q-innermost layouts so every
    operand AP has a packed 2-byte last dim -> DVE 2x mode.
  - MLP: h1_T = relu(W1.T @ [x_T; agg_T] + b1) feature-major; h2 node-major
    via h1_T-stationary matmuls into a PSUM group buffer of 4 tiles; b2 is
    added by one rank-1 ones-matmul per group.
  - LayerNorm per 4-tile PSUM group: one chunked bn_stats + 4 bn_aggr,
    rstd via ACT Rsqrt, then per tile one ACT op u = rstd*v - mu*rstd
    reading PSUM directly.  gamma-mult is one DVE 2x op per 7-tile IO
    group; the residual add (x + beta folded on host, bf16) is one GpSimd
    op per IO group.  Output is stored node-major bf16.
  - Per-core tile processing order is chosen (descending chunk count) so
    one SPMD program (a common per-tile chunk schedule) fits all cores.
"""

import os
import sys

import numpy as np

for _p in ("/opt/trn_rl_repo", "/root/.axon_site/_ro/trn_rl_repo"):
    if os.path.isdir(_p) and _p not in sys.path:
        sys.path.insert(0, _p)

import ml_dtypes

import concourse.bacc as bacc
import concourse.bass as bass
import concourse.tile as tile
from concourse import mybir
from concourse.bass_utils import run_bass_kernel_spmd

BF16 = ml_dtypes.bfloat16
FP8 = ml_dtypes.float8_e3m4

if os.environ.get("KERNEL_LDW_OPT"):
    from concourse import bass_utils as _bu

    _orig_run_command = _bu.run_command

    def _patched_run_command(argv, **kw):
        argv = [
            "--enable-ldw-opt=true" if a == "--enable-ldw-opt=false" else a
            for a in argv
        ]
        return _orig_run_command(argv, **kw)

    _bu.run_command = _patched_run_command

N_NODES = 50000
N_EDGES = 600000
D = 128           # node/edge feature dim
H = 256           # hidden dim
NCORE = 8
NSHARD = N_NODES // NCORE      # 6250 real nodes per core
P = 128                        # partition / tile size
NT = 49                        # node tiles per core (49*128 = 6272 >= 6250)
G = 7                          # IO tile group size (NT = G*G)
LG = 4                         # LayerNorm PSUM group size (tiles per bank)
NPAD = NT * P                  # padded nodes per core
L = 32                         # edge chunks per DMA load
W = 32                         # scatter window width (max cross-core span 27)
SB = 16                        # windows per batched S-build op
LN_EPS = 1e-5
PAD_J = 200.0                  # j_rel sentinel for padded edge rows


def _prep_host(x, edge_index, edge_attr, W1, b1, W2, b2, ln_g, ln_b):
    """Sort/shard/pack all inputs."""
    j = np.asarray(edge_index[1], dtype=np.int64)
    perm = np.argsort(j, kind="stable")
    js = j[perm]

    edge_attr_q = np.asarray(edge_attr, dtype=FP8)
    x = np.asarray(x, dtype=np.float32)
    ln_b = np.asarray(ln_b, dtype=np.float32)

    bounds = np.searchsorted(js, np.arange(NCORE + 1) * NSHARD)

    core_info = []
    for c in range(NCORE):
        es, ee = bounds[c], bounds[c + 1]
        jl = js[es:ee] - c * NSHARD           # local node id, 0..6249
        rows = perm[es:ee]                    # rows into edge_attr
        cnt = np.bincount(jl // P, minlength=NT)  # edges per tile
        ch = -(-cnt // P)                     # ceil chunks per tile
        tile_perm = np.argsort(-ch, kind="stable")  # descending chunk count
        core_info.append((jl, rows, cnt, ch, tile_perm))

    sorted_ch = np.stack([ci[3][ci[4]] for ci in core_info])  # [NCORE, NT]
    schedule = np.maximum(sorted_ch.max(axis=0), 1).astype(np.int64)
    nchunk = int(schedule.sum())
    nload = -(-nchunk // L)
    nc_tot = nload * L

    chunk_base = np.zeros(NT + 1, dtype=np.int64)
    np.cumsum(schedule, out=chunk_base[1:])

    # Tile-relative j_rel per chunk slot per core; chunk 0 of a tile is
    # full-width, later chunks use a common W-wide window.
    minj = np.full((NCORE, nc_tot), 1 << 30, dtype=np.int64)
    maxj = np.full((NCORE, nc_tot), -1, dtype=np.int64)
    per_core_fill = []
    for c in range(NCORE):
        jl, rows, cnt, ch, tile_perm = core_info[c]
        tile_start = np.zeros(NT + 1, dtype=np.int64)
        np.cumsum(cnt, out=tile_start[1:])
        ridx = np.zeros(nc_tot * P, dtype=np.int64)
        jrel_t = np.full(nc_tot * P, -1, dtype=np.int64)  # tile-relative
        for s in range(NT):
            T = int(tile_perm[s])
            n = int(cnt[T])
            dst = chunk_base[s] * P
            ridx[dst : dst + n] = rows[tile_start[T] : tile_start[T] + n]
            jrel_t[dst : dst + n] = jl[tile_start[T] : tile_start[T] + n] - T * P
        jr2 = jrel_t.reshape(nc_tot, P)
        valid = jr2 >= 0
        anyv = valid.any(axis=1)
        mn = np.where(anyv, np.where(valid, jr2, 1 << 30).min(axis=1), 1 << 30)
        mx = np.where(anyv, np.where(valid, jr2, -1).max(axis=1), -1)
        minj[c] = mn
        maxj[c] = mx
        per_core_fill.append((ridx, jrel_t))

    woff = np.clip(minj.min(axis=0), 0, P - W)
    woff[chunk_base[:-1]] = 0  # chunk 0 full width
    fw = np.zeros(nc_tot, dtype=bool)
    fw[chunk_base[:-1]] = True
    width = np.where(fw, P, W)
    assert (maxj.max(axis=0) < woff + width).all(), "chunk span exceeds window"

    in_maps = []
    for c in range(NCORE):
        jl, rows, cnt, ch, tile_perm = core_info[c]
        ridx, jrel_t = per_core_fill[c]
        jr2 = jrel_t.reshape(nc_tot, P).astype(np.float32) - woff[:, None]
        jr2[jrel_t.reshape(nc_tot, P) < 0] = PAD_J

        ea_all = edge_attr_q[ridx]            # [nc_tot*P, D] fp8
        ea_pack = (
            ea_all.reshape(nload, L, P, D)
            .transpose(0, 2, 1, 3)
            .reshape(nload, P, L * D)
            .copy()
        )
        jr_pack = np.ascontiguousarray(jr2.T.astype(BF16))  # [P, nc_tot]
        # chunk-0 columns (tile-relative j_rel) gathered into slot order
        jr0_pack = np.ascontiguousarray(jr2[chunk_base[:-1]].T.astype(BF16))

        # x shard: fp8 feature-major tiles (MLP input) and bf16 node-major
        # residual (+ beta folded), ordered by tile_perm, G tiles per DMA.
        xs = np.zeros((NPAD, D), dtype=np.float32)
        xs[:NSHARD] = x[c * NSHARD : (c + 1) * NSHARD]
        xt = xs.reshape(NT, P, D).transpose(0, 2, 1)[tile_perm]  # [NT, f, n]
        # MLP input packed in LG-quads (h1 batches 4 tiles per weight load)
        NQ = -(-NT // LG)
        xtq = np.zeros((NQ * LG, D, P), dtype=np.float32)
        xtq[:NT] = xt
        xbf_pack = (
            xtq.astype(FP8).reshape(NQ, LG, D, P).transpose(0, 2, 1, 3)
            .reshape(NQ, D, LG * P).copy()
        )
        xfn = (xs + ln_b[None, :]).reshape(NT, P, D)[tile_perm]  # [NT, n, f]
        xf_pack = (
            xfn.astype(BF16).reshape(G, G, P, D).transpose(0, 2, 1, 3)
            .reshape(G, P, G * D).copy()
        )

        vecs = np.asarray(b1, np.float32).reshape(H, 1)

        in_maps.append(
            {
                "ea": ea_pack,
                "jr": jr_pack,
                "jr0": jr0_pack,
                "xbf": xbf_pack,
                "xf": xf_pack,
                "W1d": np.asarray(W1, BF16),
                "W2d": np.asarray(W2, BF16),
                "vecs": vecs,
                "b2g": np.tile(np.asarray(b2, BF16).reshape(1, D), (1, LG)),
                "gb": np.tile(np.asarray(ln_g, np.float32), (P, 1)).astype(BF16),
                # iota consts with the batch index innermost (packed stride-1
                # last dims on every operand -> DVE 2x mode)
                "iotaw_q": np.tile(
                    np.repeat(np.arange(W, dtype=np.float32), SB), (P, 1)
                ).astype(BF16),
                "iotag_t": np.tile(
                    np.repeat(np.arange(P, dtype=np.float32), G), (P, 1)
                ).astype(BF16),
            }
        )

    b2_zero = bool(np.all(np.asarray(b2) == 0))
    meta = (schedule, woff, nload, nc_tot, b2_zero)
    return in_maps, meta, [ci[4] for ci in core_info]


def _build_program(meta):
    schedule, woff, nload, nc_tot, b2_zero = meta
    f32 = mybir.dt.float32
    bf16 = mybir.dt.bfloat16
    fp8 = mybir.dt.float8e3
    AF = mybir.ActivationFunctionType
    OP = mybir.AluOpType

    nc = bacc.Bacc("TRN2", target_bir_lowering=False, debug=False,
                   num_devices=NCORE)

    ea_d = nc.dram_tensor("ea", [nload, P, L * D], fp8, kind="ExternalInput").ap()
    jr_d = nc.dram_tensor("jr", [P, nc_tot], bf16, kind="ExternalInput").ap()
    jr0_d = nc.dram_tensor("jr0", [P, NT], bf16, kind="ExternalInput").ap()
    NQ = -(-NT // LG)
    xbf_d = nc.dram_tensor("xbf", [NQ, D, LG * P], fp8, kind="ExternalInput").ap()
    xf_d = nc.dram_tensor("xf", [G, P, G * D], bf16, kind="ExternalInput").ap()
    w1_d = nc.dram_tensor("W1d", [H, H], bf16, kind="ExternalInput").ap()
    w2_d = nc.dram_tensor("W2d", [H, D], bf16, kind="ExternalInput").ap()
    vecs_d = nc.dram_tensor("vecs", [H, 1], f32, kind="ExternalInput").ap()
    b2g_d = nc.dram_tensor("b2g", [1, LG * D], bf16, kind="ExternalInput").ap()
    gb_d = nc.dram_tensor("gb", [P, D], bf16, kind="ExternalInput").ap()
    iotaw_d = nc.dram_tensor("iotaw_q", [P, W * SB], bf16, kind="ExternalInput").ap()
    iotag_d = nc.dram_tensor("iotag_t", [P, P * G], bf16, kind="ExternalInput").ap()
    out_d = nc.dram_tensor("outN", [G, P, G * D], bf16, kind="ExternalOutput").ap()

    with tile.TileContext(nc) as tc:
        with (
            tc.tile_pool(name="consts", bufs=1) as consts,
            tc.tile_pool(name="edges", bufs=6) as epool,
            tc.tile_pool(name="xg", bufs=2) as xpool,
            tc.tile_pool(name="yg", bufs=2) as ypool,
            tc.tile_pool(name="s0", bufs=3) as s0pool,
            tc.tile_pool(name="sm", bufs=18) as spool,
            tc.tile_pool(name="work", bufs=3) as wpool,
            tc.tile_pool(name="ln", bufs=2) as lnpool,
            tc.tile_pool(name="ps", bufs=1, space="PSUM") as pspool,
            tc.tile_pool(name="ps2", bufs=3, space="PSUM") as ps2pool,
            tc.tile_pool(name="psagg", bufs=3, space="PSUM") as psagg,
        ):
            # ---- constants ----
            jr_sb = consts.tile([P, nc_tot], bf16)
            nc.sync.dma_start(out=jr_sb[:], in_=jr_d[:])
            jr0_sb = consts.tile([P, NT], bf16, tag="jr0")
            nc.sync.dma_start(out=jr0_sb[:], in_=jr0_d[:])
            iotaw_sb = consts.tile([P, W * SB], bf16, tag="iotaw")
            nc.sync.dma_start(out=iotaw_sb[:], in_=iotaw_d[:])
            iotag_sb = consts.tile([P, P * G], bf16, tag="iotag")
            nc.sync.dma_start(out=iotag_sb[:], in_=iotag_d[:])
            gb_sb = consts.tile([P, D], bf16, tag="gb")
            nc.sync.dma_start(out=gb_sb[:], in_=gb_d[:])
            b2g_sb = consts.tile([1, LG * D], bf16, tag="b2g")
            nc.sync.dma_start(out=b2g_sb[:], in_=b2g_d[:])
            ones_row = consts.tile([1, P], bf16, tag="ones_row")
            nc.vector.memset(ones_row[:], 1.0)

            w1xa = consts.tile([P, P], bf16, tag="w1xa")
            nc.sync.dma_start(out=w1xa[:], in_=w1_d[0:P, 0:P])
            w1xb = consts.tile([P, P], bf16, tag="w1xb")
            nc.sync.dma_start(out=w1xb[:], in_=w1_d[0:P, P : 2 * P])
            w1ga = consts.tile([P, P], bf16, tag="w1ga")
            nc.sync.dma_start(out=w1ga[:], in_=w1_d[P : 2 * P, 0:P])
            w1gb = consts.tile([P, P], bf16, tag="w1gb")
            nc.sync.dma_start(out=w1gb[:], in_=w1_d[P : 2 * P, P : 2 * P])
            w2a = consts.tile([P, P], bf16, tag="w2a")
            nc.sync.dma_start(out=w2a[:], in_=w2_d[0:P, :])
            w2b = consts.tile([P, P], bf16, tag="w2b")
            nc.sync.dma_start(out=w2b[:], in_=w2_d[P : 2 * P, :])

            b1a = consts.tile([P, 1], f32, tag="b1a")
            nc.sync.dma_start(out=b1a[:], in_=vecs_d[0:P, :])
            b1b = consts.tile([P, 1], f32, tag="b1b")
            nc.sync.dma_start(out=b1b[:], in_=vecs_d[P : 2 * P, :])
            eps_sb = consts.tile([P, 1], f32, tag="eps")
            nc.vector.memset(eps_sb[:], LN_EPS)

            def mid_bcast(a, shape):
                """AP broadcasting a [P, k] slice to [P, shape[1], k]."""
                return bass.AP(
                    tensor=a.tensor, offset=a.offset,
                    ap=[a.ap[0], [0, shape[1]], a.ap[1]],
                )

            load_tiles = {}

            def ensure_load(ld):
                if ld < 0 or ld >= nload or ld in load_tiles:
                    return
                et = epool.tile([P, L * D], fp8, tag="ea", name=f"ea{ld}")
                nc.sync.dma_start(out=et[:], in_=ea_d[ld])
                load_tiles[ld] = et

            def edge_slice(c):
                ld, sl = divmod(c, L)
                ensure_load(ld)
                ensure_load(ld + 1)
                ensure_load(ld + 2)
                return load_tiles[ld][:, sl * D : (sl + 1) * D]

            chunk_base = np.zeros(NT + 1, dtype=np.int64)
            np.cumsum(schedule, out=chunk_base[1:])

            # batched full-width S for the chunk-0s of one IO tile group,
            # layout [e, n, t] (t innermost -> 2x mode)
            s0_tiles = {}

            def s0_group(gi):
                if gi not in s0_tiles:
                    S0g = s0pool.tile([P, P * G], bf16, tag="S0g")
                    jr0s = jr0_sb[:, gi * G : (gi + 1) * G]
                    nc.vector.tensor_tensor(
                        out=S0g[:].rearrange("p (n t) -> p n t", t=G),
                        in0=mid_bcast(jr0s, [P, P, G]),
                        in1=iotag_sb[:].rearrange("p (n t) -> p n t", t=G),
                        op=OP.is_equal,
                    )
                    s0_tiles[gi] = S0g
                return s0_tiles[gi]

            def s0_rhs(gi, ti):
                S0g = s0_group(gi)
                a = S0g[:]
                return bass.AP(tensor=a.tensor, offset=a.offset + ti,
                               ap=[a.ap[0], [G, P]])

            aggT_pairs = {}
            s_of = {}

            def sbuild_tile(t):
                """Selection matrices for tile t, layout [e, w, q]."""
                c0 = int(chunk_base[t])
                ncch = int(schedule[t])
                s0_group(t // G)
                sbs = []
                for q0 in range(1, ncch, SB):
                    qn = min(SB, ncch - q0)
                    Sb = spool.tile([P, W * SB], bf16, tag="Sb",
                                    name=f"Sb{t}_{q0}")
                    jrs = jr_sb[:, c0 + q0 : c0 + q0 + qn]
                    nc.vector.tensor_tensor(
                        out=Sb[:, : W * qn].rearrange("p (w q) -> p w q", q=qn),
                        in0=mid_bcast(jrs, [P, W, qn]),
                        in1=bass.AP(tensor=iotaw_sb.tensor,
                                    offset=iotaw_sb[:].offset,
                                    ap=[iotaw_sb[:].ap[0], [SB, W], [1, qn]]),
                        op=OP.is_equal,
                    )
                    sbs.append((Sb, qn))
                s_of[t] = sbs

            def win_rhs(Sb, qn, i):
                a = Sb[:]
                return bass.AP(tensor=a.tensor, offset=a.offset + i,
                               ap=[a.ap[0], [qn, W]])

            def scatter_tile(t):
                gi, ti = divmod(t, G)
                c0 = int(chunk_base[t])
                ncch = int(schedule[t])
                agg_ps = psagg.tile([P, P], f32, tag="agg")
                nc.tensor.matmul(
                    agg_ps[:], lhsT=edge_slice(c0), rhs=s0_rhs(gi, ti),
                    start=True, stop=(ncch == 1),
                )
                sbs = s_of.pop(t)
                for bi, q0 in enumerate(range(1, ncch, SB)):
                    Sb, qn = sbs[bi]
                    for i in range(qn):
                        c = c0 + q0 + i
                        w = int(woff[c])
                        nc.tensor.matmul(
                            agg_ps[:, w : w + W],
                            lhsT=edge_slice(c),
                            rhs=win_rhs(Sb, qn, i),
                            start=False,
                            stop=(c == c0 + ncch - 1),
                            skip_group_check=True,
                        )
                # copy to SBUF so the PSUM bank frees early; quads of tiles
                # share one SBUF tile so h1 can batch over all four
                p, half = divmod(t, LG)
                if half == 0:
                    aggT_pairs[p] = wpool.tile([P, LG * P], bf16, tag="aggT",
                                               name=f"aggT{p}")
                # PSUM->SBUF cast: GpSimd cannot read PSUM, so split the
                # copies between ACT and DVE to balance engine load
                dst = aggT_pairs[p][:, half * P : (half + 1) * P]
                if t % 2 == 0:
                    nc.scalar.activation(out=dst, in_=agg_ps[:],
                                         func=AF.Copy, bias=0.0, scale=1.0)
                else:
                    nc.vector.tensor_copy(out=dst, in_=agg_ps[:])

            group_res = {}

            def group_tiles(gi):
                if gi not in group_res:
                    xf_g = xpool.tile([P, G * D], bf16, tag="xf")
                    nc.sync.dma_start(out=xf_g[:], in_=xf_d[gi])
                    u_g = ypool.tile([P, G * D], bf16, tag="ug")
                    group_res[gi] = (xf_g, u_g)
                return group_res[gi]

            def mlp_h1_quad(p):
                """h1 for tiles 4p..4p+3 batched over the node axis."""
                t0 = LG * p
                nt = min(LG, NT - t0)
                xb_q = xpool.tile([P, LG * P], fp8, tag="xb",
                                  name=f"xb{p}")
                nc.sync.dma_start(out=xb_q[:], in_=xbf_d[p])
                aggT = aggT_pairs.pop(p)
                NN = nt * P
                xT = xb_q[:, 0:NN]

                h1a_ps = pspool.tile([P, LG * P], f32, tag="h1a")
                nc.tensor.matmul(h1a_ps[:, 0:NN], lhsT=w1xa[:], rhs=xT,
                                 start=True, stop=False)
                nc.tensor.matmul(h1a_ps[:, 0:NN], lhsT=w1ga[:],
                                 rhs=aggT[:, 0:NN], start=False, stop=True)
                h1a = wpool.tile([P, LG * P], bf16, tag="h1a_sb")
                nc.scalar.activation(out=h1a[:, 0:NN], in_=h1a_ps[:, 0:NN],
                                     func=AF.Relu, bias=b1a[:], scale=1.0)

                h1b_ps = pspool.tile([P, LG * P], f32, tag="h1b")
                nc.tensor.matmul(h1b_ps[:, 0:NN], lhsT=w1xb[:], rhs=xT,
                                 start=True, stop=False)
                nc.tensor.matmul(h1b_ps[:, 0:NN], lhsT=w1gb[:],
                                 rhs=aggT[:, 0:NN], start=False, stop=True)
                h1b = wpool.tile([P, LG * P], bf16, tag="h1b_sb")
                nc.scalar.activation(out=h1b[:, 0:NN], in_=h1b_ps[:, 0:NN],
                                     func=AF.Relu, bias=b1b[:], scale=1.0)
                return h1a, h1b

            # ---- h2 into a 4-tile PSUM group, LN tail per group ----
            ln_state = {}

            def h2_tile(t, h1a, h1b, half):
                lg, li = divmod(t, LG)
                if li == 0:
                    ln_state[lg] = ps2pool.tile([P, LG * P], f32, tag="h2g",
                                                name=f"h2g{lg}")
                h2g = ln_state[lg]
                sl = slice(li * P, (li + 1) * P)
                # start=True clears the has_written bits of the whole PSUM
                # BANK, so only the group's first matmul may set it; later
                # slices rely on the bank-wide clear (first write with
                # start=False overwrites where has_written=0)
                nc.tensor.matmul(h2g[:, sl],
                                 lhsT=h1a[:, half * P : (half + 1) * P],
                                 rhs=w2a[:], start=(li == 0), stop=False,
                                 skip_group_check=(li != 0))
                last = (li == LG - 1) or (t == NT - 1)
                nc.tensor.matmul(h2g[:, sl],
                                 lhsT=h1b[:, half * P : (half + 1) * P],
                                 rhs=w2b[:], start=False,
                                 stop=(b2_zero and last),
                                 skip_group_check=True)

            def ln_group(lg):
                """b2 + LayerNorm scalars for tiles [4*lg, 4*lg+nt)."""
                t0 = lg * LG
                nt = min(LG, NT - t0)
                h2g = ln_state[lg]
                NN = nt * P
                # rank-1 b2 add over the whole group, closes all accum
                # groups.  Skipped when b2 == 0 (a 1-row weight load is
                # incompatible with the walrus LDW/FWL optimization).
                if not b2_zero:
                    nc.tensor.matmul(h2g[:, 0:NN], lhsT=ones_row[:],
                                     rhs=b2g_sb[:, 0:NN], start=False,
                                     stop=True, skip_group_check=True)
                # PSUM -> SBUF copy (exact: Copy with float bias), stats on
                # the SBUF copy, normalize on GpSimd (LUT-free)
                v_g = lnpool.tile([P, LG * P], bf16, tag="v_g")
                stats = lnpool.tile([P, LG * 6], f32, tag="stats")
                mv = lnpool.tile([P, LG * 2], f32, tag="mv")
                for i in range(nt):
                    nc.scalar.activation(out=v_g[:, i * P : (i + 1) * P],
                                         in_=h2g[:, i * P : (i + 1) * P],
                                         func=AF.Copy, bias=0.0, scale=1.0)
                    nc.vector.bn_stats(
                        out=stats[:, 6 * i : 6 * i + 6],
                        in_=v_g[:, i * P : (i + 1) * P],
                    )
                    nc.vector.bn_aggr(out=mv[:, 2 * i : 2 * i + 2],
                                      in_=stats[:, 6 * i : 6 * i + 6])
                mva = mv[:]
                var_sl = bass.AP(tensor=mva.tensor, offset=mva.offset + 1,
                                 ap=[mva.ap[0], [2, nt]])
                rstd = lnpool.tile([P, LG], f32, tag="rstd")
                nc.scalar.activation(out=rstd[:, 0:nt], in_=var_sl,
                                     func=AF.Sqrt, bias=eps_sb[:], scale=1.0)
                nc.vector.reciprocal(out=rstd[:, 0:nt], in_=rstd[:, 0:nt])
                # u = (v - mu) * rstd per tile on GpSimd
                for i in range(nt):
                    t = t0 + i
                    gi, ti = divmod(t, G)
                    _, u_g = group_tiles(gi)
                    nc.vector.tensor_scalar(
                        out=u_g[:, ti * D : (ti + 1) * D],
                        in0=v_g[:, i * P : (i + 1) * P],
                        scalar1=mv[:, 2 * i : 2 * i + 1],
                        scalar2=rstd[:, i : i + 1],
                        op0=OP.subtract, op1=OP.mult,
                    )
                del ln_state[lg]

            def io_group(gi):
                """gamma mult + residual add + store for IO group gi."""
                xf_g, u_g = group_res[gi]
                gba = gb_sb[:]
                t2 = ypool.tile([P, G * D], bf16, tag="t2")
                nc.gpsimd.tensor_tensor(
                    out=t2[:].rearrange("p (t f) -> p t f", f=D),
                    in0=u_g[:].rearrange("p (t f) -> p t f", f=D),
                    in1=bass.AP(tensor=gba.tensor, offset=gba.offset,
                                ap=[gba.ap[0], [0, G], [1, D]]),
                    op=OP.mult,
                )
                y_g = ypool.tile([P, G * D], bf16, tag="yg")
                nc.gpsimd.tensor_tensor(out=y_g[:], in0=t2[:], in1=xf_g[:],
                                        op=OP.add)
                nc.sync.dma_start(out=out_d[gi], in_=y_g[:])
                del group_res[gi]

            # software pipeline: S-builds run SA tiles ahead of the scatter
            # matmuls, which run MA tiles ahead of the MLP
            SA, MA = 16, 6
            for t in range(min(SA, NT)):
                sbuild_tile(t)
            for t in range(min(MA, NT)):
                scatter_tile(t)
            nquads = -(-NT // LG)
            next_io = 0
            for p in range(nquads):
                t0 = LG * p
                quad = [t for t in range(t0, t0 + LG) if t < NT]
                for t in quad:
                    if t + SA < NT:
                        sbuild_tile(t + SA)
                h1a, h1b = mlp_h1_quad(p)
                for t in quad:
                    if t + MA < NT:
                        scatter_tile(t + MA)
                for t in quad:
                    h2_tile(t, h1a, h1b, t - t0)
                    if t % LG == LG - 1 or t == NT - 1:
                        ln_group(t // LG)
                        # an IO group may only fire once every LN group
                        # covering it has produced its u values
                        while (next_io * G + G - 1 <= t
                               and next_io < (NT + G - 1) // G):
                            io_group(next_io)
                            next_io += 1

    nc.finalize()
    return nc


LAST_RESULT = None


def kernel(x, edge_index, edge_attr, W1, b1, W2, b2, ln_g, ln_b):
    global LAST_RESULT
    in_maps, meta, tile_perms = _prep_host(
        x, edge_index, edge_attr, W1, b1, W2, b2, ln_g, ln_b
    )
    nc = _build_program(meta)
    trace = bool(os.environ.get("KERNEL_TRACE"))
    res = run_bass_kernel_spmd(
        nc, in_maps, core_ids=list(range(NCORE)), trace=trace
    )
    LAST_RESULT = res

    out = np.empty((N_NODES, D), dtype=np.float32)
    for c in range(NCORE):
        yN = np.asarray(res.results[c]["outN"], dtype=np.float32)
        y_slots = yN.reshape(G, P, G, D).transpose(0, 2, 1, 3).reshape(NT, P, D)
        y_tiles = np.empty_like(y_slots)
        y_tiles[tile_perms[c]] = y_slots
        y = y_tiles.reshape(NPAD, D)[:NSHARD]
        out[c * NSHARD : (c + 1) * NSHARD] = y
    return out



# revision 4
# speedup vs baseline: 1.0663x; 1.0663x over previous
"""Trainium2 Bass kernel for nn_NodeProcessor (GNN message passing), v2.

Strategy (8 NeuronCores, SPMD, no collectives):
  - Host sorts edges by destination node and shards NODES (6250/core);
    each core receives exactly the edges destined to its node shard, so no
    cross-core reduction is needed.
  - On device, segment-sum is computed per 128-node tile as a sequence of
    128-edge-chunk matmuls accumulating in PSUM (one-hot S matrices built
    on DVE by is_equal against iota constants; chunk 0 full width, later
    chunks a W=32 window at a host-baked offset).
  - Edge payload and the x MLP input are fp8 e3m4 (halves HBM traffic).
  - MLP: h1_T = relu(W1.T @ [x_T; agg_T] + b1) feature-major; h2 node-major
    via h1_T-stationary matmuls into a PSUM group buffer of LG=4 tiles.
  - LayerNorm per LG group directly on PSUM: one batched bn_stats + per-
    tile bn_aggr, rstd via ACT Sqrt + DVE reciprocal, apply as ACT
    Identity(in*rstd - mu*rstd) reading PSUM.  gamma-mult + residual-add
    (x + beta folded on host, bf16) on GpSimd per group; store per group.

v2 structural changes vs v1 (trace-driven):
  - All small constants packed into 2 bf16 + 1 f32 host tensors -> 3 DMA
    issues instead of 14 (each HWDGE dma_start costs ~610ns of sequencer).
  - DMA issue split across both HWDGE rings: Sync(SP) carries edge loads +
    output stores; Scalar(ACT) carries consts/xbf/xf.  xbf and xf are each
    ONE load instead of 13/7.
  - PE warm-up: ~28 dummy matmuls with zero deps issued first so the HAM
    clock-gate un-throttles before the real stream starts.
  - LN tail restructured: no PSUM->SBUF copy (stats+apply read PSUM),
    output stores per LG group (smaller tail), batched bn_stats.
"""

import os
import sys

import numpy as np

for _p in ("/opt/trn_rl_repo", "/root/.axon_site/_ro/trn_rl_repo"):
    if os.path.isdir(_p) and _p not in sys.path:
        sys.path.insert(0, _p)

import ml_dtypes

import concourse.bacc as bacc
import concourse.bass as bass
import concourse.tile as tile
from concourse import mybir
from concourse.bass_utils import run_bass_kernel_spmd

BF16 = ml_dtypes.bfloat16
FP8 = ml_dtypes.float8_e3m4

if os.environ.get("KERNEL_LDW_OPT"):
    from concourse import bass_utils as _bu

    _orig_run_command = _bu.run_command

    def _patched_run_command(argv, **kw):
        argv = [
            "--enable-ldw-opt=true" if a == "--enable-ldw-opt=false" else a
            for a in argv
        ]
        return _orig_run_command(argv, **kw)

    _bu.run_command = _patched_run_command

N_NODES = 50000
N_EDGES = 600000
D = 128           # node/edge feature dim
H = 256           # hidden dim
NCORE = 8
NSHARD = N_NODES // NCORE      # 6250 real nodes per core
P = 128                        # partition / tile size
NT = 49                        # node tiles per core (49*128 = 6272 >= 6250)
G = 7                          # S0-build batch size (NT = G*G)
LG = 4                         # LN/store group size (tiles per PSUM bank)
NQ = -(-NT // LG)              # number of LG groups (13)
NPAD = NT * P                  # padded nodes per core
L = 32                         # edge chunks per DMA load
W = 32                         # scatter window width (max cross-core span 27)
SB = 16                        # windows per batched S-build op
LN_EPS = 1e-5
PAD_J = 200.0                  # j_rel sentinel for padded edge rows
N_WARMUP = 28                  # HAM warm-up matmuls


def _prep_host(x, edge_index, edge_attr, W1, b1, W2, b2, ln_g, ln_b):
    """Sort/shard/pack all inputs."""
    j = np.asarray(edge_index[1], dtype=np.int64)
    perm = np.argsort(j, kind="stable")
    js = j[perm]

    edge_attr_q = np.asarray(edge_attr, dtype=FP8)
    x = np.asarray(x, dtype=np.float32)
    ln_b = np.asarray(ln_b, dtype=np.float32)

    bounds = np.searchsorted(js, np.arange(NCORE + 1) * NSHARD)

    core_info = []
    for c in range(NCORE):
        es, ee = bounds[c], bounds[c + 1]
        jl = js[es:ee] - c * NSHARD           # local node id, 0..6249
        rows = perm[es:ee]                    # rows into edge_attr
        cnt = np.bincount(jl // P, minlength=NT)  # edges per tile
        ch = -(-cnt // P)                     # ceil chunks per tile
        tile_perm = np.argsort(-ch, kind="stable")  # descending chunk count
        core_info.append((jl, rows, cnt, ch, tile_perm))

    sorted_ch = np.stack([ci[3][ci[4]] for ci in core_info])  # [NCORE, NT]
    schedule = np.maximum(sorted_ch.max(axis=0), 1).astype(np.int64)
    nchunk = int(schedule.sum())
    nload = -(-nchunk // L)
    nc_tot = nload * L

    chunk_base = np.zeros(NT + 1, dtype=np.int64)
    np.cumsum(schedule, out=chunk_base[1:])

    # Tile-relative j_rel per chunk slot per core; chunk 0 of a tile is
    # full-width, later chunks use a common W-wide window.
    minj = np.full((NCORE, nc_tot), 1 << 30, dtype=np.int64)
    maxj = np.full((NCORE, nc_tot), -1, dtype=np.int64)
    per_core_fill = []
    for c in range(NCORE):
        jl, rows, cnt, ch, tile_perm = core_info[c]
        tile_start = np.zeros(NT + 1, dtype=np.int64)
        np.cumsum(cnt, out=tile_start[1:])
        ridx = np.zeros(nc_tot * P, dtype=np.int64)
        jrel_t = np.full(nc_tot * P, -1, dtype=np.int64)  # tile-relative
        for s in range(NT):
            T = int(tile_perm[s])
            n = int(cnt[T])
            dst = chunk_base[s] * P
            ridx[dst : dst + n] = rows[tile_start[T] : tile_start[T] + n]
            jrel_t[dst : dst + n] = jl[tile_start[T] : tile_start[T] + n] - T * P
        jr2 = jrel_t.reshape(nc_tot, P)
        valid = jr2 >= 0
        anyv = valid.any(axis=1)
        mn = np.where(anyv, np.where(valid, jr2, 1 << 30).min(axis=1), 1 << 30)
        mx = np.where(anyv, np.where(valid, jr2, -1).max(axis=1), -1)
        minj[c] = mn
        maxj[c] = mx
        per_core_fill.append((ridx, jrel_t))

    woff = np.clip(minj.min(axis=0), 0, P - W)
    woff[chunk_base[:-1]] = 0  # chunk 0 full width
    fw = np.zeros(nc_tot, dtype=bool)
    fw[chunk_base[:-1]] = True
    width = np.where(fw, P, W)
    assert (maxj.max(axis=0) < woff + width).all(), "chunk span exceeds window"

    b2_zero = bool(np.all(np.asarray(b2) == 0))

    in_maps = []
    for c in range(NCORE):
        jl, rows, cnt, ch, tile_perm = core_info[c]
        ridx, jrel_t = per_core_fill[c]
        jr2 = jrel_t.reshape(nc_tot, P).astype(np.float32) - woff[:, None]
        jr2[jrel_t.reshape(nc_tot, P) < 0] = PAD_J

        ea_all = edge_attr_q[ridx]            # [nc_tot*P, D] fp8
        ea_pack = (
            ea_all.reshape(nload, L, P, D)
            .transpose(0, 2, 1, 3)
            .reshape(nload, P, L * D)
            .copy()
        )
        jr_pack = np.ascontiguousarray(jr2.T.astype(BF16))  # [P, nc_tot]
        # chunk-0 columns (tile-relative j_rel) gathered into slot order
        jr0_pack = np.ascontiguousarray(jr2[chunk_base[:-1]].T.astype(BF16))
        iotaw = np.tile(
            np.repeat(np.arange(W, dtype=np.float32), SB), (P, 1)
        ).astype(BF16)
        iotag = np.tile(
            np.repeat(np.arange(P, dtype=np.float32), G), (P, 1)
        ).astype(BF16)
        # const pack A (S-build deps): jr | jr0 | iotaw | iotag
        cbfA = np.concatenate([jr_pack, jr0_pack, iotaw, iotag], axis=1)

        # const pack B (MLP deps): gb | W1 quads | W2 halves
        gb = np.tile(np.asarray(ln_g, np.float32), (P, 1)).astype(BF16)
        W1b = np.asarray(W1, BF16)
        W2b = np.asarray(W2, BF16)
        cbfB = np.concatenate(
            [gb,
             W1b[0:P, 0:P], W1b[0:P, P:2*P],
             W1b[P:2*P, 0:P], W1b[P:2*P, P:2*P],
             W2b[0:P, :], W2b[P:2*P, :]],
            axis=1,
        )
        cf32 = np.ascontiguousarray(
            np.asarray(b1, np.float32).reshape(2, P).T
        )  # [P, 2]: col0=b1[:128], col1=b1[128:]

        # x shard: fp8 feature-major (MLP input) and bf16 node-major
        # residual (+ beta folded), both in tile_perm slot order.
        xs = np.zeros((NPAD, D), dtype=np.float32)
        xs[:NSHARD] = x[c * NSHARD : (c + 1) * NSHARD]
        xt = xs.reshape(NT, P, D).transpose(0, 2, 1)[tile_perm]  # [NT, f, n]
        xtq = np.zeros((NQ * LG, D, P), dtype=np.float32)
        xtq[:NT] = xt
        # one tensor [D, NQ*LG*P], quad-major cols
        xbf_pack = np.ascontiguousarray(
            xtq.astype(FP8).transpose(1, 0, 2).reshape(D, NQ * LG * P)
        )
        xfn = (xs + ln_b[None, :]).reshape(NT, P, D)[tile_perm]  # [NT, n, f]
        xf_pack = np.ascontiguousarray(
            xfn.astype(BF16).transpose(1, 0, 2).reshape(P, NT * D)
        )

        m = {
            "ea": ea_pack,
            "cbfA": cbfA,
            "cbfB": cbfB,
            "cf32": cf32,
            "xbf": xbf_pack,
            "xf": xf_pack,
        }
        if not b2_zero:
            m["b2g"] = np.tile(np.asarray(b2, BF16).reshape(1, D), (1, LG))
        in_maps.append(m)

    meta = (schedule, woff, nload, nc_tot, b2_zero)
    return in_maps, meta, [ci[4] for ci in core_info]


def _build_program(meta):
    schedule, woff, nload, nc_tot, b2_zero = meta
    f32 = mybir.dt.float32
    bf16 = mybir.dt.bfloat16
    fp8 = mybir.dt.float8e3
    AF = mybir.ActivationFunctionType
    OP = mybir.AluOpType

    nc = bacc.Bacc("TRN2", target_bir_lowering=False, debug=False,
                   num_devices=NCORE)

    NCA = nc_tot + NT + W * SB + P * G
    NCB = D + 6 * P
    ea_d = nc.dram_tensor("ea", [nload, P, L * D], fp8, kind="ExternalInput").ap()
    cbfA_d = nc.dram_tensor("cbfA", [P, NCA], bf16, kind="ExternalInput").ap()
    cbfB_d = nc.dram_tensor("cbfB", [P, NCB], bf16, kind="ExternalInput").ap()
    cf32_d = nc.dram_tensor("cf32", [P, 2], f32, kind="ExternalInput").ap()
    xbf_d = nc.dram_tensor("xbf", [D, NQ * LG * P], fp8, kind="ExternalInput").ap()
    xf_d = nc.dram_tensor("xf", [P, NT * D], bf16, kind="ExternalInput").ap()
    if not b2_zero:
        b2g_d = nc.dram_tensor("b2g", [1, LG * D], bf16, kind="ExternalInput").ap()
    out_d = nc.dram_tensor("outN", [NQ, P, LG * D], bf16, kind="ExternalOutput").ap()

    with tile.TileContext(nc) as tc:
        with (
            tc.tile_pool(name="consts", bufs=1) as consts,
            tc.tile_pool(name="edges", bufs=6) as epool,
            tc.tile_pool(name="xg", bufs=2) as xpool,
            tc.tile_pool(name="yg", bufs=3) as ypool,
            tc.tile_pool(name="s0", bufs=3) as s0pool,
            tc.tile_pool(name="sm", bufs=18) as spool,
            tc.tile_pool(name="work", bufs=3) as wpool,
            tc.tile_pool(name="ln", bufs=3) as lnpool,
            tc.tile_pool(name="ps", bufs=1, space="PSUM") as pspool,
            tc.tile_pool(name="ps2", bufs=3, space="PSUM") as ps2pool,
            tc.tile_pool(name="psagg", bufs=3, space="PSUM") as psagg,
        ):
            # ---- PE warm-up: zero-dep matmul stream to lift the HAM gate
            # (shares the psagg ring; its bank is recycled by scatter tile 2)
            wz = consts.tile([P, P], bf16, tag="wz")
            nc.vector.memset(wz[:], 0.0)
            wups = psagg.tile([P, P], f32, tag="agg")
            for i in range(N_WARMUP):
                nc.tensor.matmul(wups[:], lhsT=wz[:], rhs=wz[:],
                                 start=(i == 0), stop=(i == N_WARMUP - 1))

            # ---- constants (scalar=ACT HWDGE ring) ----
            cA = consts.tile([P, NCA], bf16, tag="cA")
            nc.scalar.dma_start(out=cA[:], in_=cbfA_d[:])
            cB = consts.tile([P, NCB], bf16, tag="cB")
            nc.scalar.dma_start(out=cB[:], in_=cbfB_d[:])
            cf = consts.tile([P, 2], f32, tag="cf")
            nc.scalar.dma_start(out=cf[:], in_=cf32_d[:])
            xbf_sb = consts.tile([D, NQ * LG * P], fp8, tag="xbf")
            nc.scalar.dma_start(out=xbf_sb[:], in_=xbf_d[:])
            xf_sb = consts.tile([P, NT * D], bf16, tag="xf")
            nc.scalar.dma_start(out=xf_sb[:], in_=xf_d[:])

            o = 0
            jr_sb = cA[:, o:o + nc_tot]; o += nc_tot
            jr0_sb = cA[:, o:o + NT]; o += NT
            iotaw_sb = cA[:, o:o + W * SB]; o += W * SB
            iotag_sb = cA[:, o:o + P * G]
            o = 0
            gb_sb = cB[:, o:o + D]; o += D
            w1xa = cB[:, o:o + P]; o += P
            w1xb = cB[:, o:o + P]; o += P
            w1ga = cB[:, o:o + P]; o += P
            w1gb = cB[:, o:o + P]; o += P
            w2a = cB[:, o:o + P]; o += P
            w2b = cB[:, o:o + P]
            b1a = cf[:, 0:1]
            b1b = cf[:, 1:2]

            eps_sb = consts.tile([P, 1], f32, tag="eps")
            nc.vector.memset(eps_sb[:], LN_EPS)
            if not b2_zero:
                ones_row = consts.tile([1, P], bf16, tag="ones_row")
                nc.vector.memset(ones_row[:], 1.0)
                b2g_sb = consts.tile([1, LG * D], bf16, tag="b2g")
                nc.scalar.dma_start(out=b2g_sb[:], in_=b2g_d[:])

            def mid_bcast(a, shape):
                """AP broadcasting a [P, k] slice to [P, shape[1], k]."""
                return bass.AP(
                    tensor=a.tensor, offset=a.offset,
                    ap=[a.ap[0], [0, shape[1]], a.ap[1]],
                )

            load_tiles = {}

            def ensure_load(ld):
                if ld < 0 or ld >= nload or ld in load_tiles:
                    return
                et = epool.tile([P, L * D], fp8, tag="ea", name=f"ea{ld}")
                nc.sync.dma_start(out=et[:], in_=ea_d[ld])
                load_tiles[ld] = et

            def edge_slice(c):
                ld, sl = divmod(c, L)
                ensure_load(ld)
                ensure_load(ld + 1)
                ensure_load(ld + 2)
                return load_tiles[ld][:, sl * D : (sl + 1) * D]

            chunk_base = np.zeros(NT + 1, dtype=np.int64)
            np.cumsum(schedule, out=chunk_base[1:])

            # batched full-width S for the chunk-0s of G tiles,
            # layout [e, n, t] (t innermost -> 2x mode)
            s0_tiles = {}

            def s0_group(gi):
                if gi not in s0_tiles:
                    S0g = s0pool.tile([P, P * G], bf16, tag="S0g")
                    jr0s = jr0_sb[:, gi * G : (gi + 1) * G]
                    nc.vector.tensor_tensor(
                        out=S0g[:].rearrange("p (n t) -> p n t", t=G),
                        in0=mid_bcast(jr0s, [P, P, G]),
                        in1=iotag_sb.rearrange("p (n t) -> p n t", t=G),
                        op=OP.is_equal,
                    )
                    s0_tiles[gi] = S0g
                return s0_tiles[gi]

            def s0_rhs(gi, ti):
                S0g = s0_group(gi)
                a = S0g[:]
                return bass.AP(tensor=a.tensor, offset=a.offset + ti,
                               ap=[a.ap[0], [G, P]])

            aggT_pairs = {}
            s_of = {}

            def sbuild_tile(t):
                """Selection matrices for tile t, layout [e, w, q]."""
                c0 = int(chunk_base[t])
                ncch = int(schedule[t])
                s0_group(t // G)
                sbs = []
                for q0 in range(1, ncch, SB):
                    qn = min(SB, ncch - q0)
                    Sb = spool.tile([P, W * SB], bf16, tag="Sb",
                                    name=f"Sb{t}_{q0}")
                    jrs = jr_sb[:, c0 + q0 : c0 + q0 + qn]
                    nc.vector.tensor_tensor(
                        out=Sb[:, : W * qn].rearrange("p (w q) -> p w q", q=qn),
                        in0=mid_bcast(jrs, [P, W, qn]),
                        in1=bass.AP(tensor=iotaw_sb.tensor,
                                    offset=iotaw_sb.offset,
                                    ap=[iotaw_sb.ap[0], [SB, W], [1, qn]]),
                        op=OP.is_equal,
                    )
                    sbs.append((Sb, qn))
                s_of[t] = sbs

            def win_rhs(Sb, qn, i):
                a = Sb[:]
                return bass.AP(tensor=a.tensor, offset=a.offset + i,
                               ap=[a.ap[0], [qn, W]])

            def scatter_tile(t):
                gi, ti = divmod(t, G)
                c0 = int(chunk_base[t])
                ncch = int(schedule[t])
                agg_ps = psagg.tile([P, P], f32, tag="agg")
                nc.tensor.matmul(
                    agg_ps[:], lhsT=edge_slice(c0), rhs=s0_rhs(gi, ti),
                    start=True, stop=(ncch == 1),
                )
                sbs = s_of.pop(t)
                for bi, q0 in enumerate(range(1, ncch, SB)):
                    Sb, qn = sbs[bi]
                    for i in range(qn):
                        c = c0 + q0 + i
                        w = int(woff[c])
                        nc.tensor.matmul(
                            agg_ps[:, w : w + W],
                            lhsT=edge_slice(c),
                            rhs=win_rhs(Sb, qn, i),
                            start=False,
                            stop=(c == c0 + ncch - 1),
                            skip_group_check=True,
                        )
                # copy to SBUF so the PSUM bank frees early; quads of tiles
                # share one SBUF tile so h1 can batch over all four.
                # GpSimd cannot read PSUM; split copies between ACT and DVE.
                p, half = divmod(t, LG)
                if half == 0:
                    aggT_pairs[p] = wpool.tile([P, LG * P], bf16, tag="aggT",
                                               name=f"aggT{p}")
                dst = aggT_pairs[p][:, half * P : (half + 1) * P]
                if t % 2 == 0:
                    nc.scalar.activation(out=dst, in_=agg_ps[:],
                                         func=AF.Copy, bias=0.0, scale=1.0)
                else:
                    nc.vector.tensor_copy(out=dst, in_=agg_ps[:])

            def mlp_h1_quad(p):
                """h1 for tiles 4p..4p+3 batched over the node axis."""
                t0 = LG * p
                nt = min(LG, NT - t0)
                aggT = aggT_pairs.pop(p)
                NN = nt * P
                xT = xbf_sb[:, p * LG * P : p * LG * P + NN]

                h1a_ps = pspool.tile([P, LG * P], f32, tag="h1a")
                nc.tensor.matmul(h1a_ps[:, 0:NN], lhsT=w1xa, rhs=xT,
                                 start=True, stop=False)
                nc.tensor.matmul(h1a_ps[:, 0:NN], lhsT=w1ga,
                                 rhs=aggT[:, 0:NN], start=False, stop=True)
                h1a = wpool.tile([P, LG * P], bf16, tag="h1a_sb")
                nc.scalar.activation(out=h1a[:, 0:NN], in_=h1a_ps[:, 0:NN],
                                     func=AF.Relu, bias=b1a, scale=1.0)

                h1b_ps = pspool.tile([P, LG * P], f32, tag="h1b")
                nc.tensor.matmul(h1b_ps[:, 0:NN], lhsT=w1xb, rhs=xT,
                                 start=True, stop=False)
                nc.tensor.matmul(h1b_ps[:, 0:NN], lhsT=w1gb,
                                 rhs=aggT[:, 0:NN], start=False, stop=True)
                h1b = wpool.tile([P, LG * P], bf16, tag="h1b_sb")
                nc.scalar.activation(out=h1b[:, 0:NN], in_=h1b_ps[:, 0:NN],
                                     func=AF.Relu, bias=b1b, scale=1.0)
                return h1a, h1b

            # ---- h2 into a 4-tile PSUM group, LN tail per group ----
            ln_state = {}

            def h2_tile(t, h1a, h1b, half):
                lg, li = divmod(t, LG)
                if li == 0:
                    ln_state[lg] = ps2pool.tile([P, LG * P], f32, tag="h2g",
                                                name=f"h2g{lg}")
                h2g = ln_state[lg]
                sl = slice(li * P, (li + 1) * P)
                # start=True clears the has_written bits of the whole PSUM
                # BANK, so only the group's first matmul may set it; later
                # slices rely on the bank-wide clear (first write with
                # start=False overwrites where has_written=0)
                nc.tensor.matmul(h2g[:, sl],
                                 lhsT=h1a[:, half * P : (half + 1) * P],
                                 rhs=w2a, start=(li == 0), stop=False,
                                 skip_group_check=(li != 0))
                last = (li == LG - 1) or (t == NT - 1)
                nc.tensor.matmul(h2g[:, sl],
                                 lhsT=h1b[:, half * P : (half + 1) * P],
                                 rhs=w2b, start=False,
                                 stop=(b2_zero and last),
                                 skip_group_check=True)

            def ln_group(lg):
                """b2 + LayerNorm + gamma + residual + store, tiles
                [4*lg, 4*lg+nt)."""
                t0 = lg * LG
                nt = min(LG, NT - t0)
                h2g = ln_state[lg]
                NN = nt * P
                # rank-1 b2 add over the whole group, closes all accum
                # groups.  Skipped when b2 == 0.
                if not b2_zero:
                    nc.tensor.matmul(h2g[:, 0:NN], lhsT=ones_row[:],
                                     rhs=b2g_sb[:, 0:NN], start=False,
                                     stop=True, skip_group_check=True)
                # LN stats directly on PSUM: one batched bn_stats, per-tile
                # bn_aggr
                stats = lnpool.tile([P, LG * 6], f32, tag="stats")
                mv = lnpool.tile([P, LG * 2], f32, tag="mv")
                for i in range(nt):
                    nc.vector.bn_stats(out=stats[:, 6 * i : 6 * i + 6],
                                       in_=h2g[:, i * P : (i + 1) * P])
                    nc.vector.bn_aggr(out=mv[:, 2 * i : 2 * i + 2],
                                      in_=stats[:, 6 * i : 6 * i + 6])
                mva = mv[:]
                var_sl = bass.AP(tensor=mva.tensor, offset=mva.offset + 1,
                                 ap=[mva.ap[0], [2, nt]])
                mean_sl = bass.AP(tensor=mva.tensor, offset=mva.offset,
                                  ap=[mva.ap[0], [2, nt]])
                rstd = lnpool.tile([P, LG], f32, tag="rstd")
                nc.scalar.activation(out=rstd[:, 0:nt], in_=var_sl,
                                     func=AF.Sqrt, bias=eps_sb[:], scale=1.0)
                nc.vector.reciprocal(out=rstd[:, 0:nt], in_=rstd[:, 0:nt])
                # nmr = -mu * rstd  (bias for the ACT Identity apply)
                nmr = lnpool.tile([P, LG], f32, tag="nmr")
                nc.vector.tensor_tensor(out=nmr[:, 0:nt], in0=mean_sl,
                                        in1=rstd[:, 0:nt], op=OP.mult)
                nc.vector.tensor_scalar(out=nmr[:, 0:nt], in0=nmr[:, 0:nt],
                                        scalar1=-1.0, scalar2=None,
                                        op0=OP.mult)
                # u = v*rstd - mu*rstd per tile on ACT, reading PSUM
                u_g = lnpool.tile([P, LG * D], bf16, tag="u_g")
                for i in range(nt):
                    nc.scalar.activation(
                        out=u_g[:, i * D : (i + 1) * D],
                        in_=h2g[:, i * P : (i + 1) * P],
                        func=AF.Identity,
                        bias=nmr[:, i : i + 1],
                        scale=rstd[:, i : i + 1],
                    )
                del ln_state[lg]
                # gamma mult + residual add on GpSimd, store on sync ring
                t2 = ypool.tile([P, LG * D], bf16, tag="t2")
                gba = gb_sb
                nc.gpsimd.tensor_tensor(
                    out=t2[:, 0:NN].rearrange("p (t f) -> p t f", f=D),
                    in0=u_g[:, 0:NN].rearrange("p (t f) -> p t f", f=D),
                    in1=bass.AP(tensor=gba.tensor, offset=gba.offset,
                                ap=[gba.ap[0], [0, nt], [1, D]]),
                    op=OP.mult,
                )
                y_g = ypool.tile([P, LG * D], bf16, tag="yg")
                nc.gpsimd.tensor_tensor(out=y_g[:, 0:NN], in0=t2[:, 0:NN],
                                        in1=xf_sb[:, t0 * D : t0 * D + NN],
                                        op=OP.add)
                nc.sync.dma_start(out=out_d[lg][:, 0:NN], in_=y_g[:, 0:NN])

            # software pipeline: S-builds run SA tiles ahead of the scatter
            # matmuls, which run MA tiles ahead of the MLP
            SA, MA = 16, 6
            for t in range(min(SA, NT)):
                sbuild_tile(t)
            for t in range(min(MA, NT)):
                scatter_tile(t)
            for p in range(NQ):
                t0 = LG * p
                quad = [t for t in range(t0, t0 + LG) if t < NT]
                for t in quad:
                    if t + SA < NT:
                        sbuild_tile(t + SA)
                h1a, h1b = mlp_h1_quad(p)
                for t in quad:
                    if t + MA < NT:
                        scatter_tile(t + MA)
                for t in quad:
                    h2_tile(t, h1a, h1b, t - t0)
                ln_group(p)

    nc.finalize()
    return nc


LAST_RESULT = None


def kernel(x, edge_index, edge_attr, W1, b1, W2, b2, ln_g, ln_b):
    global LAST_RESULT
    in_maps, meta, tile_perms = _prep_host(
        x, edge_index, edge_attr, W1, b1, W2, b2, ln_g, ln_b
    )
    nc = _build_program(meta)
    trace = bool(os.environ.get("KERNEL_TRACE"))
    res = run_bass_kernel_spmd(
        nc, in_maps, core_ids=list(range(NCORE)), trace=trace
    )
    LAST_RESULT = res

    out = np.empty((N_NODES, D), dtype=np.float32)
    for c in range(NCORE):
        yN = np.asarray(res.results[c]["outN"], dtype=np.float32)
        y_slots = (
            yN.reshape(NQ, P, LG, D).transpose(0, 2, 1, 3)
            .reshape(NQ * LG, P, D)[:NT]
        )
        y_tiles = np.empty_like(y_slots)
        y_tiles[tile_perms[c]] = y_slots
        y = y_tiles.reshape(NPAD, D)[:NSHARD]
        out[c * NSHARD : (c + 1) * NSHARD] = y
    return out
